# revision 4
# baseline (speedup 1.0000x reference)
"""Trainium2 Bass kernel for the chain-graph GNN message-passing module.

Strategy (graph/sequence parallel, per sharding hint):
- Shard the node/time axis T=131072 across 8 NeuronCores (16384 cols each).
- Host slices inputs with a 1-column halo on each side (the chain graph only
  needs 1-hop neighbors), precomputes the measurement-difference features
  (inv), packs/transposes the small weights, and folds the measurement
  encoder (fc_init) into the up-edge MLP so `hy` never materializes.
- On device everything stays channel-major [128 partitions, T_shard free]:
  every weight matmul is a 128x128 (or K<=17) stationary x [128,512] moving
  f32r matmul into PSUM; neighbor features are just shifted SBUF views.
- Edge MLPs: fc1 of the 3 edge types accumulates (center + shifted + attr)
  matmuls into one 3-segment PSUM tile; attr matmuls carry the fc1 biases
  via an appended ones-row (K=17); one wide Lrelu drains PSUM->SBUF.
  fc2 likewise, with biases added via K=1 ones-row matmuls.
- BatchNorm (training-mode, global over T) is computed with per-tile
  bn_stats records, per-core bn_aggr, and a tiny [128,3] AllReduce across
  the 8 cores (twice: node_mlp BN and decoder BN).
- GRU gates use PSUM accumulation of the two matmuls + fused Sigmoid/Tanh
  on the scalar engine; the decoder re-reads h_out from DRAM after the
  second AllReduce and applies BN+Relu+final 4-channel matmul.
"""
import sys

sys.path.insert(0, "/opt/trn_rl_repo")
import numpy as np

N_CORES = 8
NF, DS, DM, T = 128, 4, 2, 131072
SH = T // N_CORES          # columns per core
FD = 512                   # inner tile width
NT = SH // FD              # tiles per core

_CACHE = {}


def _build_program():
    import concourse.bacc as bacc
    import concourse.tile as tile
    import concourse.mybir as mybir

    f32 = mybir.dt.float32
    f32r = mybir.dt.float32r
    Act = mybir.ActivationFunctionType
    Op = mybir.AluOpType

    nc = bacc.Bacc("TRN2", target_bir_lowering=False, debug=False,
                   num_devices=N_CORES)

    # ---- external I/O ----
    hs_d = nc.dram_tensor("hs", [NF, SH + 2], f32r, kind="ExternalInput").ap()
    eas_d = nc.dram_tensor("eas", [17, SH], f32r, kind="ExternalInput").ap()
    wmat_d = nc.dram_tensor("wmat", [NF, 16 * 128 + 4], f32r, kind="ExternalInput").ap()
    wattr_d = nc.dram_tensor("wattr", [17, 3 * 128], f32r, kind="ExternalInput").ap()
    brow_d = nc.dram_tensor("brow", [1, 5 * 128], f32r, kind="ExternalInput").ap()
    wvec_d = nc.dram_tensor("wvec", [NF, 13], f32, kind="ExternalInput").ap()
    hout_d = nc.dram_tensor("hout", [NF, SH], f32, kind="ExternalOutput").ap()
    dec_d = nc.dram_tensor("dec", [DS, SH], f32, kind="ExternalOutput").ap()

    with tile.TileContext(nc) as tc:
        with (
            tc.tile_pool(name="const", bufs=1) as cp,
            tc.tile_pool(name="resident", bufs=1) as rp,
            tc.tile_pool(name="statsp", bufs=1) as sp,
            tc.tile_pool(name="ccdram", bufs=1, space="DRAM") as dr,
        ):
            # persistent SBUF state
            hs = rp.tile([NF, SH + 2], f32r)
            y_sb = rp.tile([NF, SH], f32)
            wmat = cp.tile([NF, 16 * 128 + 4], f32r)
            wattr = cp.tile([17, 3 * 128], f32r)
            brow = cp.tile([1, 5 * 128], f32r)
            wvec = cp.tile([NF, 13], f32)
            ones = cp.tile([1, FD], f32r)
            nc.sync.dma_start(hs[:], hs_d[:])
            nc.sync.dma_start(wmat[:], wmat_d[:])
            nc.sync.dma_start(wattr[:], wattr_d[:])
            nc.sync.dma_start(brow[:], brow_d[:])
            nc.sync.dma_start(wvec[:], wvec_d[:])
            nc.sync.dma_start(ones[:], eas_d[16:17, 0:FD])

            def W(i):
                return wmat[:, i * 128:(i + 1) * 128]

            (W1R_ROW, W1R_COL, W1U_ROW, W1L_ROW, W1L_COL,
             W2R, W2U, W2L, NODE, WIHR, WHHR, WIHZ, WHHZ,
             WIHN, WHHN, DEC1) = range(16)
            DEC2 = wmat[:, 16 * 128:16 * 128 + 4]

            def brseg(i):
                return brow[:, i * 128:(i + 1) * 128]

            B2R, B2U, B2L, BHHN, BDEC1 = range(5)

            def vcol(i):
                return wvec[:, i:i + 1]

            (NODE_B, GRU_BR, GRU_BZ, B_IHN, GAMMA1, BETA1, GAMMA2, BETA2,
             NFIXL, NFIXR, DEC2_B, DEC1_B, EPS) = range(13)

            stats1 = sp.tile([NF, 6 * NT], f32)
            stats2 = sp.tile([NF, 6 * NT], f32)
            bn1 = sp.tile([NF, 8], f32)   # scratch for bn const math
            bn2 = sp.tile([NF, 8], f32)
            a1 = sp.tile([NF, 1], f32)
            c1 = sp.tile([NF, 1], f32)
            a2 = sp.tile([NF, 1], f32)
            c2p = sp.tile([NF, 1], f32)
            cc1_s = sp.tile([NF, 3], f32)
            cc2_s = sp.tile([NF, 3], f32)

            # ---------------- Phase A: edge MLPs + node MLP pre-BN ----------------
            with (
                tc.tile_pool(name="pa_sb", bufs=2) as pa,
                tc.tile_pool(name="pa_eas", bufs=3) as pe,
                tc.tile_pool(name="pa_ps", bufs=1, space="PSUM") as pp,
                tc.tile_pool(name="pa_psy", bufs=2, space="PSUM") as ppy,
            ):
                for i in range(NT):
                    c = i * FD
                    ctr = hs[:, 1 + c:1 + c + FD]
                    lft = hs[:, c:c + FD]
                    rgt = hs[:, 2 + c:2 + c + FD]

                    eas_t = pe.tile([17, FD], f32r, tag="eas")
                    nc.sync.dma_start(eas_t[:], eas_d[:, c:c + FD])

                    pre = pp.tile([NF, 3 * FD], f32, tag="pre")
                    nc.tensor.matmul(pre[:, 0:FD], W(W1R_ROW), ctr, start=True, stop=False)
                    nc.tensor.matmul(pre[:, 0:FD], W(W1R_COL), lft, start=False, stop=False)
                    nc.tensor.matmul(pre[:, 0:FD], wattr[:, 0:128], eas_t[:], start=False, stop=True)
                    nc.tensor.matmul(pre[:, FD:2 * FD], W(W1U_ROW), ctr, start=True, stop=False)
                    nc.tensor.matmul(pre[:, FD:2 * FD], wattr[:, 128:256], eas_t[:], start=False, stop=True)
                    nc.tensor.matmul(pre[:, 2 * FD:3 * FD], W(W1L_ROW), ctr, start=True, stop=False)
                    nc.tensor.matmul(pre[:, 2 * FD:3 * FD], W(W1L_COL), rgt, start=False, stop=False)
                    nc.tensor.matmul(pre[:, 2 * FD:3 * FD], wattr[:, 256:384], eas_t[:], start=False, stop=True)

                    m_all = pa.tile([NF, 3 * FD], f32r, tag="m_all")
                    nc.scalar.activation(m_all[:], pre[:], Act.Lrelu,
                                         bias=0.0, scale=1.0, alpha=0.01)

                    fps = pp.tile([NF, 3 * FD], f32, tag="fps")
                    nc.tensor.matmul(fps[:, 0:FD], W(W2R), m_all[:, 0:FD], start=True, stop=False)
                    nc.tensor.matmul(fps[:, 0:FD], brseg(B2R), ones[:], start=False, stop=True)
                    nc.tensor.matmul(fps[:, FD:2 * FD], W(W2U), m_all[:, FD:2 * FD], start=True, stop=False)
                    nc.tensor.matmul(fps[:, FD:2 * FD], brseg(B2U), ones[:], start=False, stop=True)
                    nc.tensor.matmul(fps[:, 2 * FD:3 * FD], W(W2L), m_all[:, 2 * FD:3 * FD], start=True, stop=False)
                    nc.tensor.matmul(fps[:, 2 * FD:3 * FD], brseg(B2L), ones[:], start=False, stop=True)

                    f_all = pa.tile([NF, 3 * FD], f32, tag="f_all")
                    nc.scalar.activation(f_all[:], fps[:], Act.Lrelu,
                                         bias=0.0, scale=1.0, alpha=0.01)

                    agg = pa.tile([NF, FD], f32r, tag="agg")
                    nc.vector.tensor_add(agg[:], f_all[:, 0:FD], f_all[:, FD:2 * FD])
                    nc.vector.tensor_add(agg[:], agg[:].bitcast(f32), f_all[:, 2 * FD:3 * FD])
                    if i == 0:
                        # drop the r-edge message at global t=0 (core 0 only; NFIXL=-1 there)
                        nc.vector.scalar_tensor_tensor(
                            out=agg[:, 0:1], in0=f_all[:, 0:1],
                            scalar=vcol(NFIXL), in1=agg[:, 0:1].bitcast(f32),
                            op0=Op.mult, op1=Op.add)
                    if i == NT - 1:
                        # drop the l-edge message at global t=T-1 (core 7 only)
                        nc.vector.scalar_tensor_tensor(
                            out=agg[:, FD - 1:FD], in0=f_all[:, 3 * FD - 1:3 * FD],
                            scalar=vcol(NFIXR), in1=agg[:, FD - 1:FD].bitcast(f32),
                            op0=Op.mult, op1=Op.add)

                    psy = ppy.tile([NF, FD], f32, tag="psy")
                    nc.tensor.matmul(psy[:], W(NODE), agg[:], start=True, stop=True)
                    nc.scalar.activation(y_sb[:, c:c + FD], psy[:], Act.Identity,
                                         bias=vcol(NODE_B), scale=1.0)
                    nc.vector.bn_stats(stats1[:, 6 * i:6 * i + 6], y_sb[:, c:c + FD])

            # ---------------- BN1 stats AllReduce ----------------
            nc.vector.bn_aggr(bn1[:, 0:2], stats1[:])
            nc.vector.tensor_copy(cc1_s[:, 0:2], bn1[:, 0:2])
            nc.vector.tensor_mul(cc1_s[:, 2:3], bn1[:, 0:1], bn1[:, 0:1])
            cc1_in = dr.tile([NF, 3], f32)
            cc1_out = dr.tile([NF, 3], f32, addr_space="Shared")
            nc.sync.dma_start(cc1_in[:], cc1_s[:])
            nc.gpsimd.collective_compute(
                "AllReduce", Op.add, replica_groups=[list(range(N_CORES))],
                ins=[cc1_in[:]], outs=[cc1_out[:]])
            nc.sync.dma_start(cc1_s[:], cc1_out[:])
            # m = S_mean/8 ; v = (S_var + S_m2)/8 - m^2 ; a = gamma/sqrt(v+eps);
            # c = beta - m*a
            nc.vector.tensor_scalar_mul(bn1[:, 2:3], cc1_s[:, 0:1], 1.0 / N_CORES)   # m
            nc.vector.tensor_add(bn1[:, 3:4], cc1_s[:, 1:2], cc1_s[:, 2:3])
            nc.vector.tensor_scalar_mul(bn1[:, 3:4], bn1[:, 3:4], 1.0 / N_CORES)
            nc.vector.tensor_mul(bn1[:, 4:5], bn1[:, 2:3], bn1[:, 2:3])              # m^2
            nc.vector.tensor_sub(bn1[:, 3:4], bn1[:, 3:4], bn1[:, 4:5])              # v
            nc.scalar.activation(bn1[:, 5:6], bn1[:, 3:4], Act.Sqrt, bias=vcol(EPS), scale=1.0)
            nc.vector.reciprocal(bn1[:, 6:7], bn1[:, 5:6])                           # 1/sd
            nc.vector.tensor_mul(a1[:], wvec[:, GAMMA1:GAMMA1 + 1], bn1[:, 6:7])
            nc.vector.tensor_mul(bn1[:, 7:8], bn1[:, 2:3], a1[:])
            nc.vector.tensor_sub(c1[:], wvec[:, BETA1:BETA1 + 1], bn1[:, 7:8])

            # ---------------- Phase C: BN1 + GRU + dec1 stats ----------------
            with (
                tc.tile_pool(name="pc_sb", bufs=2) as pc,
                tc.tile_pool(name="pc_ps", bufs=1, space="PSUM") as pq,
            ):
                for i in range(NT):
                    c = i * FD
                    ctr = hs[:, 1 + c:1 + c + FD]
                    ctr_f = ctr.bitcast(f32)

                    aggbn = pc.tile([NF, FD], f32r, tag="aggbn")
                    nc.scalar.activation(aggbn[:], y_sb[:, c:c + FD], Act.Lrelu,
                                         bias=c1[:], scale=a1[:], alpha=0.01)

                    ps_r = pq.tile([NF, FD], f32, tag="ps_r")
                    nc.tensor.matmul(ps_r[:], W(WIHR), aggbn[:], start=True, stop=False)
                    nc.tensor.matmul(ps_r[:], W(WHHR), ctr, start=False, stop=True)
                    ps_z = pq.tile([NF, FD], f32, tag="ps_z")
                    nc.tensor.matmul(ps_z[:], W(WIHZ), aggbn[:], start=True, stop=False)
                    nc.tensor.matmul(ps_z[:], W(WHHZ), ctr, start=False, stop=True)
                    ps_gn = pq.tile([NF, FD], f32, tag="ps_gn")
                    nc.tensor.matmul(ps_gn[:], W(WIHN), aggbn[:], start=True, stop=True)
                    ps_ghn = pq.tile([NF, FD], f32, tag="ps_ghn")
                    nc.tensor.matmul(ps_ghn[:], W(WHHN), ctr, start=True, stop=False)
                    nc.tensor.matmul(ps_ghn[:], brseg(BHHN), ones[:], start=False, stop=True)

                    r = pc.tile([NF, FD], f32, tag="r")
                    nc.scalar.activation(r[:], ps_r[:], Act.Sigmoid, bias=vcol(GRU_BR), scale=1.0)
                    z = pc.tile([NF, FD], f32, tag="z")
                    nc.scalar.activation(z[:], ps_z[:], Act.Sigmoid, bias=vcol(GRU_BZ), scale=1.0)

                    rghn = pc.tile([NF, FD], f32, tag="rghn")
                    nc.vector.tensor_mul(rghn[:], r[:], ps_ghn[:])
                    nc.vector.tensor_add(rghn[:], ps_gn[:], rghn[:])
                    n = pc.tile([NF, FD], f32, tag="n")
                    nc.scalar.activation(n[:], rghn[:], Act.Tanh, bias=vcol(B_IHN), scale=1.0)

                    d = pc.tile([NF, FD], f32, tag="d")
                    nc.vector.tensor_sub(d[:], ctr_f, n[:])
                    nc.vector.tensor_mul(d[:], z[:], d[:])
                    hnew = pc.tile([NF, FD], f32r, tag="hnew")
                    nc.vector.tensor_add(hnew[:], n[:], d[:])
                    nc.sync.dma_start(hout_d[:, c:c + FD], hnew[:].bitcast(f32))

                    ps_x = pq.tile([NF, FD], f32, tag="ps_x")
                    nc.tensor.matmul(ps_x[:], W(DEC1), hnew[:], start=True, stop=False)
                    nc.tensor.matmul(ps_x[:], brseg(BDEC1), ones[:], start=False, stop=True)
                    nc.vector.bn_stats(stats2[:, 6 * i:6 * i + 6], ps_x[:])

            # ---------------- BN2 stats AllReduce ----------------
            nc.vector.bn_aggr(bn2[:, 0:2], stats2[:])
            nc.vector.tensor_copy(cc2_s[:, 0:2], bn2[:, 0:2])
            nc.vector.tensor_mul(cc2_s[:, 2:3], bn2[:, 0:1], bn2[:, 0:1])
            cc2_in = dr.tile([NF, 3], f32)
            cc2_out = dr.tile([NF, 3], f32, addr_space="Shared")
            nc.sync.dma_start(cc2_in[:], cc2_s[:])
            nc.gpsimd.collective_compute(
                "AllReduce", Op.add, replica_groups=[list(range(N_CORES))],
                ins=[cc2_in[:]], outs=[cc2_out[:]])
            nc.sync.dma_start(cc2_s[:], cc2_out[:])
            nc.vector.tensor_scalar_mul(bn2[:, 2:3], cc2_s[:, 0:1], 1.0 / N_CORES)
            nc.vector.tensor_add(bn2[:, 3:4], cc2_s[:, 1:2], cc2_s[:, 2:3])
            nc.vector.tensor_scalar_mul(bn2[:, 3:4], bn2[:, 3:4], 1.0 / N_CORES)
            nc.vector.tensor_mul(bn2[:, 4:5], bn2[:, 2:3], bn2[:, 2:3])
            nc.vector.tensor_sub(bn2[:, 3:4], bn2[:, 3:4], bn2[:, 4:5])
            nc.scalar.activation(bn2[:, 5:6], bn2[:, 3:4], Act.Sqrt, bias=vcol(EPS), scale=1.0)
            nc.vector.reciprocal(bn2[:, 6:7], bn2[:, 5:6])
            nc.vector.tensor_mul(a2[:], wvec[:, GAMMA2:GAMMA2 + 1], bn2[:, 6:7])
            nc.vector.tensor_mul(bn2[:, 7:8], bn2[:, 2:3], a2[:])
            nc.vector.tensor_sub(c2p[:], wvec[:, BETA2:BETA2 + 1], bn2[:, 7:8])
            # fold dec1_b into the BN bias: c2' = c2 + a2 * dec1_b
            nc.vector.tensor_mul(bn2[:, 4:5], a2[:], vcol(DEC1_B))
            nc.vector.tensor_add(c2p[:], c2p[:], bn2[:, 4:5])

            # ---------------- Phase E: decode ----------------
            with (
                tc.tile_pool(name="pe_sb", bufs=3) as pg,
                tc.tile_pool(name="pe_ps", bufs=2, space="PSUM") as ph,
            ):
                for i in range(NT):
                    c = i * FD
                    hh = pg.tile([NF, FD], f32r, tag="hh")
                    nc.sync.dma_start(hh[:], hout_d[:, c:c + FD].bitcast(f32r))
                    ps_x2 = ph.tile([NF, FD], f32, tag="ps_x2")
                    nc.tensor.matmul(ps_x2[:], W(DEC1), hh[:], start=True, stop=True)
                    xbn = pg.tile([NF, FD], f32r, tag="xbn")
                    nc.scalar.activation(xbn[:], ps_x2[:], Act.Relu,
                                         bias=c2p[:], scale=a2[:])
                    ps_d = ph.tile([NF, FD], f32, tag="ps_d")
                    nc.tensor.matmul(ps_d[0:DS, :], DEC2, xbn[:], start=True, stop=True)
                    dec_t = pg.tile([DS, FD], f32, tag="dec_t")
                    nc.vector.tensor_scalar_add(dec_t[:], ps_d[0:DS, :], wvec[0:DS, DEC2_B:DEC2_B + 1])
                    nc.sync.dma_start(dec_d[:, c:c + FD], dec_t[:])

    nc.compile()
    return nc


def _prep_in_maps(inputs):
    f = np.float32
    h = np.asarray(inputs["h"], f)[0]          # [128, T]
    meas = np.asarray(inputs["meas"], f)[0]    # [2, T]
    ea_r = np.asarray(inputs["ea_r"], f)[0]
    ea_u = np.asarray(inputs["ea_u"], f)[0]
    ea_l = np.asarray(inputs["ea_l"], f)[0]

    # measurement difference features (replicate-padded), computed on host
    left = np.concatenate([meas[:, :1], meas[:, :-1]], axis=1)
    right = np.concatenate([meas[:, 1:], meas[:, -1:]], axis=1)
    inv = np.concatenate([meas - left, right - meas], axis=0)  # [4, T]

    fc_init_w = np.asarray(inputs["fc_init_w"], f)
    fc_init_b = np.asarray(inputs["fc_init_b"], f)
    Wl1, bl1 = np.asarray(inputs["Wl1"], f), np.asarray(inputs["bl1"], f)
    Wl2, bl2 = np.asarray(inputs["Wl2"], f), np.asarray(inputs["bl2"], f)
    Wr1, br1 = np.asarray(inputs["Wr1"], f), np.asarray(inputs["br1"], f)
    Wr2, br2 = np.asarray(inputs["Wr2"], f), np.asarray(inputs["br2"], f)
    Wu1, bu1 = np.asarray(inputs["Wu1"], f), np.asarray(inputs["bu1"], f)
    Wu2, bu2 = np.asarray(inputs["Wu2"], f), np.asarray(inputs["bu2"], f)
    node_w, node_b = np.asarray(inputs["node_w"], f), np.asarray(inputs["node_b"], f)
    node_gamma = np.asarray(inputs["node_gamma"], f)
    node_beta = np.asarray(inputs["node_beta"], f)
    W_ih, W_hh = np.asarray(inputs["W_ih"], f), np.asarray(inputs["W_hh"], f)
    b_ih, b_hh = np.asarray(inputs["b_ih"], f), np.asarray(inputs["b_hh"], f)
    dec1_w, dec1_b = np.asarray(inputs["dec1_w"], f), np.asarray(inputs["dec1_b"], f)
    dec_gamma = np.asarray(inputs["dec_gamma"], f)
    dec_beta = np.asarray(inputs["dec_beta"], f)
    dec2_w, dec2_b = np.asarray(inputs["dec2_w"], f), np.asarray(inputs["dec2_b"], f)

    # wmat: 16 transposed 128x128 blocks + dec2^T
    blocks = [
        Wr1[:, 0:128].T, Wr1[:, 128:256].T, Wu1[:, 0:128].T,
        Wl1[:, 0:128].T, Wl1[:, 128:256].T,
        Wr2.T, Wu2.T, Wl2.T, node_w.T,
        W_ih[0:128].T, W_hh[0:128].T,
        W_ih[128:256].T, W_hh[128:256].T,
        W_ih[256:384].T, W_hh[256:384].T,
        dec1_w.T,
    ]
    wmat = np.concatenate(blocks + [dec2_w.T], axis=1).astype(f)  # [128, 2052]

    # attr17 blocks: rows 0-3 ea_r, 4-7 ea_u, 8-11 ea_l, 12-15 inv, 16 ones
    attr_r = np.zeros((17, 128), f)
    attr_r[0:4] = Wr1[:, 256:260].T
    attr_r[16] = br1
    attr_u = np.zeros((17, 128), f)
    attr_u[4:8] = Wu1[:, 256:260].T
    attr_u[12:16] = (Wu1[:, 128:256] @ fc_init_w).T
    attr_u[16] = bu1 + Wu1[:, 128:256] @ fc_init_b
    attr_l = np.zeros((17, 128), f)
    attr_l[8:12] = Wl1[:, 256:260].T
    attr_l[16] = bl1
    wattr = np.concatenate([attr_r, attr_u, attr_l], axis=1)  # [17, 384]

    brow = np.concatenate([
        br2, bu2, bl2, b_hh[256:384], dec1_b
    ]).astype(f)[None, :]  # [1, 640]

    in_maps = []
    for cix in range(N_CORES):
        a, b = cix * SH, (cix + 1) * SH
        lo = max(a - 1, 0)
        hi = min(b + 1, T)
        hslice = h[:, lo:hi]
        if a == 0:
            hslice = np.concatenate([h[:, 0:1], hslice], axis=1)
        if b == T:
            hslice = np.concatenate([hslice, h[:, T - 1:T]], axis=1)

        eas = np.empty((17, SH), f)
        eas[0:4] = ea_r[:, a:b]
        eas[4:8] = ea_u[:, a:b]
        eas[8:12] = ea_l[:, a:b]
        eas[12:16] = inv[:, a:b]
        eas[16] = 1.0

        wvec = np.zeros((128, 13), f)
        wvec[:, 12] = 1e-5
        wvec[:, 0] = node_b
        wvec[:, 1] = b_ih[0:128] + b_hh[0:128]
        wvec[:, 2] = b_ih[128:256] + b_hh[128:256]
        wvec[:, 3] = b_ih[256:384]
        wvec[:, 4] = node_gamma
        wvec[:, 5] = node_beta
        wvec[:, 6] = dec_gamma
        wvec[:, 7] = dec_beta
        wvec[:, 8] = -1.0 if cix == 0 else 0.0          # NFIXL
        wvec[:, 9] = -1.0 if cix == N_CORES - 1 else 0.0  # NFIXR
        wvec[0:DS, 10] = dec2_b
        wvec[:, 11] = dec1_b

        in_maps.append({
            "hs": np.ascontiguousarray(hslice),
            "eas": eas,
            "wmat": wmat,
            "wattr": wattr,
            "brow": brow,
            "wvec": wvec,
        })
    return in_maps


def _get_nc():
    if "nc" not in _CACHE:
        _CACHE["nc"] = _build_program()
    return _CACHE["nc"]


def kernel(**inputs):
    from concourse import bass_utils

    nc = _get_nc()
    in_maps = _prep_in_maps(inputs)
    res = bass_utils.run_bass_kernel_spmd(nc, in_maps, core_ids=list(range(N_CORES)))
    _CACHE["last_exec_time_ns"] = res.exec_time_ns

    h_out = np.empty((1, NF, T), np.float32)
    dec = np.empty((1, DS, T), np.float32)
    for cix in range(N_CORES):
        a, b = cix * SH, (cix + 1) * SH
        h_out[0][:, a:b] = res.results[cix]["hout"]
        dec[0][:, a:b] = res.results[cix]["dec"]
    return dec, h_out
